# revision 2
# baseline (speedup 1.0000x reference)
"""DCNv3 Trainium2 kernel: 4-core SPMD, core = batch (data-parallel over
batch, per the sharding hint). Each core runs both group-pairs
sequentially, reusing SBUF. vs the 8-core variant this halves the
number of transfer shards (per-shard tunnel latency dominates) and
removes the 2x duplication of x across group-pair cores.

Single packed bf16 input per core (padded x grid + weights, f32 consts
as bf16 hi/lo pairs); single packed bf16 output (values + GroupNorm
stats hi/lo + max-offset flags). Shifted x replicas built on device.
Exact for |offset| <= 1; host applies exact correction for larger.
"""
import os
import sys
import numpy as np
from contextlib import ExitStack

for _p in ("/opt/trn_rl_repo",):
    if _p not in sys.path:
        sys.path.insert(0, _p)

os.environ.setdefault("JAX_COMPILATION_CACHE_DIR", "/tmp/jax_kernel_cache")
os.environ.setdefault("JAX_PERSISTENT_CACHE_MIN_ENTRY_SIZE_BYTES", "0")
os.environ.setdefault("JAX_PERSISTENT_CACHE_MIN_COMPILE_TIME_SECS", "0")


def _enable_jax_cache():
    import jax
    try:
        jax.config.update("jax_compilation_cache_dir", "/tmp/jax_kernel_cache")
        jax.config.update("jax_persistent_cache_min_entry_size_bytes", 0)
        jax.config.update("jax_persistent_cache_min_compile_time_secs", 0)
    except Exception:
        pass


G, K, CG, H, W = 4, 9, 16, 128, 128
HP, WP = H + 4, W + 4
PX = HP * WP            # 17424
BASE = WP + 1           # 133
SLACK = 2 * WP + 2      # 266
PXE = PX + 2 * SLACK    # 17956
CHUNK = 484
NCH = PX // CHUNK       # 36
MMF = 484
XRW = CHUNK + 2 * BASE  # 750
EPS = 1e-5
NPIX = H * W

# packed input layout: [64, XCOLS] bf16; consts for both group-pairs
C_CB = [PXE, PXE + 260]          # per-gp cstb as 2x[64,130]
C_FH = [PXE + 520, PXE + 536]    # per-gp cstf hi [36,8]
C_FL = [PXE + 528, PXE + 544]    # per-gp cstf lo [36,8]
XCOLS = PXE + 552                # 18508
OCOLS = NPIX + 6

_CACHE = {}


def _build_nc():
    import concourse.bass as bass
    import concourse.mybir as mybir
    from concourse import bacc, tile

    f32 = mybir.dt.float32
    mdt = mybir.dt.bfloat16
    AF = mybir.ActivationFunctionType
    OP = mybir.AluOpType
    AX = mybir.AxisListType

    nc = bacc.Bacc("TRN2", target_bir_lowering=False, debug=False)
    xin = nc.dram_tensor("xin", [64, XCOLS], mdt, kind="ExternalInput")
    outp = nc.dram_tensor("outp", [64, OCOLS], mdt, kind="ExternalOutput")

    ROWS = [128, 128, 48]

    with ExitStack() as ctx:
        tc = ctx.enter_context(tile.TileContext(nc))
        ppool = ctx.enter_context(tc.tile_pool(name="psum", bufs=4,
                                               space="PSUM"))
        dpool = ctx.enter_context(tc.tile_pool(name="drsc", bufs=1,
                                               space="DRAM"))
        for gp in range(2):
          with tc.tile_pool(name=f"keep{gp}", bufs=1) as keep:
            rb = gp * 32     # row base of this group-pair's channels in xin
            sb_cb = keep.tile([128, 130], mdt, name=f"cb{gp}")
            nc.sync.dma_start(sb_cb[0:64, :],
                              xin[:, C_CB[gp]:C_CB[gp] + 130])
            nc.sync.dma_start(sb_cb[64:128, :],
                              xin[:, C_CB[gp] + 130:C_CB[gp] + 260])
            cfh = keep.tile([36, 8], mdt, name=f"cfh{gp}")
            cfl = keep.tile([36, 8], mdt, name=f"cfl{gp}")
            nc.sync.dma_start(cfh[:], xin[0:36, C_FH[gp]:C_FH[gp] + 8])
            nc.sync.dma_start(cfl[:], xin[0:36, C_FL[gp]:C_FL[gp] + 8])
            sb_cf = keep.tile([36, 8], f32, name=f"cf{gp}")
            nc.vector.tensor_tensor(sb_cf[:], cfh[:], cfl[:], OP.add)

            sb_womT = sb_cb[0:64, 0:82]
            sb_wA = [sb_cb[:, 82:98], sb_cb[:, 98:114]]
            sb_wB = sb_cb[0:48, 114:130]
            sb_bomYX = sb_cf[0:36, 0:1]
            sb_bomM = sb_cf[0:18, 1:2]
            sb_dcnb = [sb_cf[0:16, 2:3], sb_cf[0:16, 3:4]]
            sb_gnwf = [sb_cf[0:16, 4:5], sb_cf[0:16, 5:6]]
            sb_gnbf = [sb_cf[0:16, 6:7], sb_cf[0:16, 7:8]]

            vsb = [keep.tile([16, PX], mdt, tag=f"vsb{gp}{g}",
                             name=f"vsb{gp}{g}") for g in range(2)]
            moffa = keep.tile([36, 1], f32, name=f"moffa{gp}")
            nc.vector.memset(moffa[:], 0.0)

            with tc.tile_pool(name=f"p2_{gp}", bufs=2) as p2:
                for c in range(NCH):
                    q = c * CHUNK
                    xq = p2.tile([64, CHUNK + 2 * SLACK], mdt, tag="xq",
                                 name=f"xq{gp}")
                    nc.sync.dma_start(xq[:],
                                      xin[:, q:q + CHUNK + 2 * SLACK])
                    omYX = p2.tile([36, CHUNK], f32, tag="omYX",
                                   name=f"omYX{gp}")
                    omM = p2.tile([18, CHUNK], f32, tag="omM",
                                  name=f"omM{gp}")
                    for s in range(CHUNK // MMF):
                        sl = slice(SLACK + s * MMF, SLACK + (s + 1) * MMF)
                        so = slice(s * MMF, (s + 1) * MMF)
                        ps = ppool.tile([82, MMF], f32, tag="omps",
                                        name=f"omps{gp}")
                        nc.tensor.matmul(ps[:], sb_womT, xq[:, sl],
                                         start=True, stop=True)
                        nc.scalar.activation(omYX[:, so], ps[0:36, :],
                                             AF.Identity, bias=sb_bomYX)
                        nc.scalar.activation(omM[:, so], ps[64:82, :],
                                             AF.Identity, bias=sb_bomM)
                    ayx = p2.tile([36, 3, CHUNK], mdt, tag="ayx",
                                  name=f"ayx{gp}")
                    for m in range(3):
                        tmp = p2.tile([36, CHUNK], f32, tag="tmp_m",
                                      name=f"tmp{gp}")
                        tabs = p2.tile([36, CHUNK], f32, tag="tabs_m",
                                       name=f"tabs{gp}")
                        nc.vector.tensor_scalar(tmp[:], omYX[:],
                                                float(1 - m), None, OP.add)
                        nc.vector.scalar_tensor_tensor(tabs[:], tmp[:], -1.0,
                                                       tmp[:], OP.mult,
                                                       OP.max)
                        if m == 1:
                            mr = p2.tile([36, 1], f32, tag="mr",
                                         name=f"mr{gp}")
                            nc.vector.tensor_reduce(mr[:], tabs[:],
                                                    axis=AX.X, op=OP.max)
                            nc.vector.tensor_tensor(moffa[:], moffa[:],
                                                    mr[:], OP.max)
                        nc.scalar.activation(ayx[:, m, :], tabs[:], AF.Relu,
                                             bias=1.0, scale=-1.0)
                    ms = p2.tile([18, CHUNK], mdt, tag="ms", name=f"ms{gp}")
                    nc.scalar.activation(ms[:], omM[:], AF.Sigmoid)
                    axT = p2.tile([18, 3, CHUNK], mdt, tag="axT",
                                  name=f"axT{gp}")
                    nc.sync.dma_start(axT[:], ayx[18:36, :, :])
                    ayp = p2.tile([18, 3, CHUNK], mdt, tag="ayp",
                                  name=f"ayp{gp}")
                    for m in range(3):
                        nc.vector.tensor_tensor(ayp[:, m, :],
                                                ayx[0:18, m, :], ms[:],
                                                OP.mult)
                    a9p = p2.tile([18, K, CHUNK], mdt, tag="a9p",
                                  name=f"a9p{gp}")
                    for t in range(K):
                        m, n = t // 3, t % 3
                        nc.vector.tensor_tensor(a9p[:, t, :], ayp[:, m, :],
                                                axT[:, n, :], OP.mult)
                    # x replicas: row c*8+k <- xq[rb + gi*16 + c] shifted
                    xrc = [p2.tile([ROWS[i], XRW], mdt, tag=f"xrc{i}",
                                   name=f"xrc{gp}{i}") for i in range(3)]
                    for gi in range(2):
                        xv = xrc[gi][:].rearrange("(c k) w -> k c w", k=8)
                        for k in range(8):
                            ky, kx = k // 3, k % 3
                            s0 = SLACK - BASE + (ky - 1) * WP + (kx - 1)
                            nc.sync.dma_start(
                                xv[k],
                                xq[rb + gi * 16:rb + gi * 16 + 16,
                                   s0:s0 + XRW])
                    s8 = SLACK - BASE + WP + 1
                    nc.sync.dma_start(xrc[2][0:16, :],
                                      xq[rb:rb + 16, s8:s8 + XRW])
                    nc.sync.dma_start(xrc[2][16:32, :],
                                      xq[rb:rb + 16, s8:s8 + XRW])
                    nc.sync.dma_start(xrc[2][32:48, :],
                                      xq[rb + 16:rb + 32, s8:s8 + XRW])
                    # modulation replicas (log-doubling)
                    a9r = [p2.tile([ROWS[i], K, CHUNK], mdt, tag=f"a9r{i}",
                                   name=f"a9r{gp}{i}") for i in range(3)]
                    for g in range(2):
                        nc.sync.dma_start(a9r[g][0:8], a9p[g * 9:g * 9 + 8])
                        nc.sync.dma_start(a9r[g][8:16], a9r[g][0:8])
                        nc.sync.dma_start(a9r[g][16:32], a9r[g][0:16])
                        nc.sync.dma_start(a9r[g][32:64], a9r[g][0:32])
                        nc.sync.dma_start(a9r[g][64:128], a9r[g][0:64])
                        r0 = g * 32
                        nc.sync.dma_start(a9r[2][r0:r0+1],
                                          a9p[g * 9 + 8:g * 9 + 9])
                        nc.sync.dma_start(a9r[2][r0+1:r0+2],
                                          a9r[2][r0:r0+1])
                        nc.sync.dma_start(a9r[2][r0+2:r0+4],
                                          a9r[2][r0:r0+2])
                        nc.sync.dma_start(a9r[2][r0+4:r0+8],
                                          a9r[2][r0:r0+4])
                        nc.sync.dma_start(a9r[2][r0+8:r0+16],
                                          a9r[2][r0:r0+8])
                    nc.sync.dma_start(a9r[2][16:32], a9r[2][0:16])
                    val = [p2.tile([ROWS[i], CHUNK], mdt, tag=f"val{i}",
                                   name=f"val{gp}{i}") for i in range(3)]
                    for i in range(3):
                        prod = p2.tile([ROWS[i], CHUNK], mdt, tag=f"prod{i}",
                                       name=f"prod{gp}{i}")
                        for t in range(K):
                            m, n = t // 3, t % 3
                            off = BASE + (m - 1) * WP + (n - 1)
                            dst = val[i] if t == 0 else prod
                            nc.vector.tensor_tensor(
                                dst[:], a9r[i][:, t, :],
                                xrc[i][:, off:off + CHUNK], OP.mult)
                            if t > 0:
                                nc.vector.tensor_tensor(val[i][:], val[i][:],
                                                        prod[:], OP.add)
                    for g in range(2):
                        for s in range(CHUNK // MMF):
                            sl = slice(s * MMF, (s + 1) * MMF)
                            psv = ppool.tile([16, MMF], f32, tag="psv",
                                             name=f"psv{gp}")
                            nc.tensor.matmul(psv[:], sb_wA[g],
                                             val[g][:, sl],
                                             start=True, stop=False)
                            nc.tensor.matmul(psv[:], sb_wB[g*32:g*32+16, :],
                                             val[2][g*32:g*32+16, sl],
                                             start=False, stop=True)
                            nc.scalar.activation(
                                vsb[g][:, q+s*MMF:q+(s+1)*MMF],
                                psv[:], AF.Identity, bias=sb_dcnb[g])
            moffb = keep.tile([36, 1], mdt, name=f"moffb{gp}")
            nc.vector.tensor_copy(moffb[:], moffa[:])
            nc.sync.dma_start(outp[0:36, NPIX + 4 + gp:NPIX + 5 + gp],
                              moffb[:])

            # GroupNorm + Gelu for this group-pair
            with tc.tile_pool(name=f"p3_{gp}", bufs=1) as p3:
                VOFF = 2 * WP
                invN = 1.0 / (CG * NPIX)
                zero16 = p3.tile([1, 16], f32, name=f"z16{gp}")
                nc.vector.memset(zero16[:], 0.0)
                for g in range(2):
                    orow = gp * 32 + g * 16
                    vg = vsb[g]
                    vap = vg[:, VOFF:VOFF + H*WP].rearrange(
                        "p (h w) -> p h w", w=WP)[:, :, 2:2 + W]
                    vsq = p3.tile([16, PX], f32, tag="vsq",
                                  name=f"vsq{gp}")
                    nc.scalar.activation(vsq[:], vg[:], AF.Square)
                    sqap = vsq[:, VOFF:VOFF + H*WP].rearrange(
                        "p (h w) -> p h w", w=WP)[:, :, 2:2 + W]
                    r1 = p3.tile([16, H], f32, tag="r1", name=f"r1{gp}")
                    s1 = p3.tile([16, 1], f32, tag="s1", name=f"s1{gp}")
                    nc.vector.tensor_reduce(r1[:], vap, axis=AX.X, op=OP.add)
                    nc.vector.tensor_reduce(s1[:], r1[:], axis=AX.X,
                                            op=OP.add)
                    r2 = p3.tile([16, H], f32, tag="r2", name=f"r2{gp}")
                    s2 = p3.tile([16, 1], f32, tag="s2", name=f"s2{gp}")
                    nc.vector.tensor_reduce(r2[:], sqap, axis=AX.X,
                                            op=OP.add)
                    nc.vector.tensor_reduce(s2[:], r2[:], axis=AX.X,
                                            op=OP.add)
                    stats = p3.tile([16, 2], f32, tag="stats",
                                    name=f"stats{gp}")
                    nc.vector.tensor_copy(stats[:, 0:1], s1[:])
                    nc.vector.tensor_copy(stats[:, 1:2], s2[:])
                    sthi = p3.tile([16, 2], mdt, tag="sthi",
                                   name=f"sthi{gp}")
                    nc.vector.tensor_copy(sthi[:], stats[:])
                    sthf = p3.tile([16, 2], f32, tag="sthf",
                                   name=f"sthf{gp}")
                    nc.vector.tensor_copy(sthf[:], sthi[:])
                    stlf = p3.tile([16, 2], f32, tag="stlf",
                                   name=f"stlf{gp}")
                    nc.vector.tensor_tensor(stlf[:], stats[:], sthf[:],
                                            OP.subtract)
                    stlo = p3.tile([16, 2], mdt, tag="stlo",
                                   name=f"stlo{gp}")
                    nc.vector.tensor_copy(stlo[:], stlf[:])
                    nc.sync.dma_start(outp[orow:orow+16, NPIX:NPIX + 2],
                                      sthi[:])
                    nc.sync.dma_start(outp[orow:orow+16, NPIX + 2:NPIX + 4],
                                      stlo[:])
                    scr1 = dpool.tile([16, 1], f32, tag="scr1",
                                      name=f"scr1{gp}")
                    scr2 = dpool.tile([16, 1], f32, tag="scr2",
                                      name=f"scr2{gp}")
                    nc.sync.dma_start(scr1[:], s1[:])
                    nc.sync.dma_start(scr2[:], s2[:])
                    s1t = p3.tile([1, 16], f32, tag="s1t", name=f"s1t{gp}")
                    s2t = p3.tile([1, 16], f32, tag="s2t", name=f"s2t{gp}")
                    nc.sync.dma_start(s1t[:], scr1[:].rearrange("p x -> x p"))
                    nc.sync.dma_start(s2t[:], scr2[:].rearrange("p x -> x p"))
                    mug = p3.tile([1, 1], f32, tag="mug", name=f"mug{gp}")
                    e2g = p3.tile([1, 1], f32, tag="e2g", name=f"e2g{gp}")
                    nc.vector.tensor_reduce(mug[:], s1t[:], axis=AX.X,
                                            op=OP.add)
                    nc.vector.tensor_reduce(e2g[:], s2t[:], axis=AX.X,
                                            op=OP.add)
                    nc.vector.tensor_scalar(mug[:], mug[:], invN, None,
                                            OP.mult)
                    nc.vector.tensor_scalar(e2g[:], e2g[:], invN, None,
                                            OP.mult)
                    var = p3.tile([1, 1], f32, tag="var", name=f"var{gp}")
                    nc.vector.tensor_tensor(var[:], mug[:], mug[:], OP.mult)
                    nc.vector.tensor_tensor(var[:], e2g[:], var[:],
                                            OP.subtract)
                    nc.vector.tensor_scalar(var[:], var[:], EPS, None,
                                            OP.add)
                    sd = p3.tile([1, 1], f32, tag="sd", name=f"sd{gp}")
                    nc.scalar.activation(sd[:], var[:], AF.Sqrt)
                    ivg = p3.tile([1, 1], f32, tag="ivg", name=f"ivg{gp}")
                    nc.vector.reciprocal(ivg[:], sd[:])
                    inv16 = p3.tile([1, 16], f32, tag="inv16",
                                    name=f"inv16{gp}")
                    mu16 = p3.tile([1, 16], f32, tag="mu16",
                                   name=f"mu16{gp}")
                    nc.scalar.activation(inv16[:], zero16[:], AF.Identity,
                                         bias=ivg[:])
                    nc.scalar.activation(mu16[:], zero16[:], AF.Identity,
                                         bias=mug[:])
                    ivp = p3.tile([16, 1], f32, tag="ivp", name=f"ivp{gp}")
                    mup = p3.tile([16, 1], f32, tag="mup", name=f"mup{gp}")
                    scr3 = dpool.tile([1, 16], f32, tag="scr3",
                                      name=f"scr3{gp}")
                    scr4 = dpool.tile([1, 16], f32, tag="scr4",
                                      name=f"scr4{gp}")
                    nc.sync.dma_start(scr3[:], inv16[:])
                    nc.sync.dma_start(scr4[:], mu16[:])
                    nc.sync.dma_start(ivp[:], scr3[:].rearrange("x p -> p x"))
                    nc.sync.dma_start(mup[:], scr4[:].rearrange("x p -> p x"))
                    scp = p3.tile([16, 1], f32, tag="scp", name=f"scp{gp}")
                    bip = p3.tile([16, 1], f32, tag="bip", name=f"bip{gp}")
                    nc.vector.tensor_tensor(scp[:], sb_gnwf[g], ivp[:],
                                            OP.mult)
                    nc.vector.tensor_tensor(bip[:], mup[:], scp[:], OP.mult)
                    nc.vector.tensor_tensor(bip[:], sb_gnbf[g], bip[:],
                                            OP.subtract)
                    og = p3.tile([16, PX], mdt, tag="og", name=f"og{gp}")
                    nc.scalar.activation(og[:], vg[:], AF.Gelu,
                                         bias=bip[:], scale=scp[:])
                    ogap = og[:, VOFF:VOFF + H*WP].rearrange(
                        "p (h w) -> p h w", w=WP)[:, :, 2:2 + W]
                    nc.sync.dma_start(
                        outp[orow:orow+16, 0:NPIX].rearrange(
                            "p (h w) -> p h w", w=W),
                        ogap)

    if not nc.is_finalized():
        nc.finalize()
    return nc


def get_nc():
    _enable_jax_cache()
    if "nc" not in _CACHE:
        nc = _build_nc()
        _json = nc.to_json_bytes()
        nc.to_json_bytes = lambda: _json
        _CACHE["nc"] = nc
    return _CACHE["nc"]


def _consts_for_gp(w_om, b_om, dcn_w, dcn_b, gn_w, gn_b, sc, gp):
    import ml_dtypes
    bf16 = ml_dtypes.bfloat16
    idx_oy = [g * 27 + 2 * k for g in range(G) for k in range(K)]
    idx_ox = [g * 27 + 2 * k + 1 for g in range(G) for k in range(K)]
    idx_ml = [g * 27 + 18 + k for g in range(G) for k in range(K)]
    gsel = [2 * gp, 2 * gp + 1]
    cols = []
    for idx in (idx_oy, idx_ox, idx_ml):
        for g in gsel:
            cols += idx[g * K:(g + 1) * K]
    wsel = w_om[cols].astype(np.float32).copy()
    bsel = b_om[cols].astype(np.float32).copy()
    wsel[:36] *= sc
    bsel[:36] *= sc
    cb = np.zeros((128, 130), np.float32)
    cb[0:64, 0:82] = np.concatenate(
        [wsel[0:36], np.zeros((28, 64), np.float32), wsel[36:54]]).T
    for gi in range(2):
        wg = dcn_w[gsel[gi]].reshape(CG, CG, K)
        cb[:, 82+16*gi:98+16*gi] = np.transpose(
            wg[:, :, :8], (1, 2, 0)).reshape(128, CG)
        cb[gi*32:gi*32+16, 114:130] = wg[:, :, 8].T
    cf = np.zeros((36, 8), np.float32)
    cf[0:36, 0] = bsel[0:36]
    cf[0:18, 1] = bsel[36:54]
    cf[0:16, 2] = dcn_b[gsel[0]]
    cf[0:16, 3] = dcn_b[gsel[1]]
    c0 = gsel[0] * CG
    cf[0:16, 4] = gn_w[c0:c0+16]
    cf[0:16, 5] = gn_w[c0+16:c0+32]
    cf[0:16, 6] = gn_b[c0:c0+16]
    cf[0:16, 7] = gn_b[c0+16:c0+32]
    cf_hi = cf.astype(bf16)
    cf_lo = (cf - cf_hi.astype(np.float32)).astype(bf16)
    return cb.astype(bf16), cf_hi, cf_lo


def _host_prep(x, w_om, b_om, dcn_w, dcn_b, gn_w, gn_b, offset_scale):
    import ml_dtypes
    bf16 = ml_dtypes.bfloat16
    B = x.shape[0]
    sc = float(np.asarray(offset_scale).reshape(-1)[0])
    grid = np.zeros((B, 64, HP, WP), np.float32)
    grid[:, :, 2:2 + H, 2:2 + W] = x
    xpe = np.zeros((B, 64, PXE), np.float32)
    xpe[:, :, SLACK:SLACK + PX] = grid.reshape(B, 64, PX)
    xpe_b = xpe.astype(bf16)
    consts = [_consts_for_gp(w_om, b_om, dcn_w, dcn_b, gn_w, gn_b, sc, gp)
              for gp in range(2)]
    in_maps = []
    for b in range(B):
        xin = np.zeros((64, XCOLS), bf16)
        xin[:, 0:PXE] = xpe_b[b]
        for gp in range(2):
            cb, cf_hi, cf_lo = consts[gp]
            xin[:, C_CB[gp]:C_CB[gp] + 130] = cb[0:64]
            xin[:, C_CB[gp] + 130:C_CB[gp] + 260] = cb[64:128]
            xin[0:36, C_FH[gp]:C_FH[gp] + 8] = cf_hi
            xin[0:36, C_FL[gp]:C_FL[gp] + 8] = cf_lo
        in_maps.append({"xin": xin})
    return in_maps


def kernel(x, w_om, b_om, dcn_w, dcn_b, gn_w, gn_b, offset_scale):
    from concourse.bass_utils import run_bass_kernel_spmd

    x = np.asarray(x, np.float32)
    w_om = np.asarray(w_om, np.float32)
    b_om = np.asarray(b_om, np.float32)
    dcn_w = np.asarray(dcn_w, np.float32)
    dcn_b = np.asarray(dcn_b, np.float32)
    gn_w = np.asarray(gn_w, np.float32)
    gn_b = np.asarray(gn_b, np.float32)
    offset_scale = np.asarray(offset_scale, np.float32)
    in_maps = _host_prep(x, w_om, b_om, dcn_w, dcn_b, gn_w, gn_b,
                         offset_scale)
    nc = get_nc()
    res = run_bass_kernel_spmd(nc, in_maps, core_ids=[0, 1, 2, 3])
    out = np.zeros((4, 64, H, W), np.float32)
    stats = np.zeros((4, 64, 2), np.float32)
    moff_all = 0.0
    for b in range(4):
        r = res.results[b]["outp"].astype(np.float32)
        out[b] = r[:, 0:NPIX].reshape(64, H, W)
        stats[b] = r[:, NPIX:NPIX + 2] + r[:, NPIX + 2:NPIX + 4]
        moff_all = max(moff_all, float(r[0:36, NPIX + 4].max()),
                       float(r[0:36, NPIX + 5].max()))
    if moff_all > 0.99:
        out = _host_correct(out, stats, x, w_om, b_om, dcn_w, dcn_b,
                            gn_w, gn_b, offset_scale)
    return out


def _host_correct(out, stats, x, w_om, b_om, dcn_w, dcn_b, gn_w, gn_b,
                  offset_scale):
    """Exact fix for rare pixels with |offset| > 1 (clamped-tri mismatch)."""
    from scipy.special import erf, expit
    sc = float(np.asarray(offset_scale).reshape(-1)[0])
    B = x.shape[0]
    om = (np.einsum('bcp,oc->bop', x.reshape(B, 64, NPIX), w_om)
          + b_om[None, :, None]).reshape(B, 108, H, W)
    for b in range(B):
        for g in range(G):
            oy = om[b, g*27:g*27+18:2] * sc
            ox = om[b, g*27+1:g*27+18:2] * sc
            bad = (np.abs(oy) > 1).any(0) | (np.abs(ox) > 1).any(0)
            if not bad.any():
                continue
            ml = expit(om[b, g*27+18:g*27+27])
            N = CG * NPIX
            mu = stats[b, g*16:g*16+16, 0].sum() / N
            var = stats[b, g*16:g*16+16, 1].sum() / N - mu * mu
            inv = 1.0 / np.sqrt(var + EPS)
            wg = dcn_w[g].reshape(CG, CG, K)
            for hh, ww in zip(*np.nonzero(bad)):
                val = np.zeros((CG, K), np.float32)
                for k in range(K):
                    ky, kx = k // 3, k % 3
                    py = hh + ky - 1 + oy[k, hh, ww]
                    pxx = ww + kx - 1 + ox[k, hh, ww]
                    y0, x0 = int(np.floor(py)), int(np.floor(pxx))
                    fy, fx = py - y0, pxx - x0
                    acc = np.zeros(CG, np.float32)
                    for dy, wy in ((0, 1 - fy), (1, fy)):
                        for dx, wx in ((0, 1 - fx), (1, fx)):
                            yy, xx = y0 + dy, x0 + dx
                            if 0 <= yy < H and 0 <= xx < W:
                                acc += wy * wx * x[b, g*CG:g*CG+CG, yy, xx]
                    val[:, k] = acc * ml[k, hh, ww]
                pre = np.einsum('ck,ock->o', val, wg) + dcn_b[g]
                z = ((pre - mu) * inv * gn_w[g*CG:g*CG+CG]
                     + gn_b[g*CG:g*CG+CG])
                out[b, g*CG:g*CG+CG, hh, ww] = (
                    z * 0.5 * (1.0 + erf(z / np.sqrt(2.0))))
    return out


# revision 6
# speedup vs baseline: 1.0316x; 1.0316x over previous
"""DCNv3 Trainium2 kernel: 4-core SPMD, core = batch (data-parallel over
batch, per the sharding hint). Each core runs both group-pairs
sequentially, reusing SBUF. vs the 8-core variant this halves the
number of transfer shards (per-shard tunnel latency dominates) and
removes the 2x duplication of x across group-pair cores.

Single packed bf16 input per core (padded x grid + weights, f32 consts
as bf16 hi/lo pairs); single packed bf16 output (values + GroupNorm
stats hi/lo + max-offset flags). Shifted x replicas built on device.
Exact for |offset| <= 1; host applies exact correction for larger.
"""
import os
import sys
import numpy as np
from contextlib import ExitStack

for _p in ("/opt/trn_rl_repo",):
    if _p not in sys.path:
        sys.path.insert(0, _p)

os.environ.setdefault("JAX_COMPILATION_CACHE_DIR", "/tmp/jax_kernel_cache")
os.environ.setdefault("JAX_PERSISTENT_CACHE_MIN_ENTRY_SIZE_BYTES", "0")
os.environ.setdefault("JAX_PERSISTENT_CACHE_MIN_COMPILE_TIME_SECS", "0")


def _enable_jax_cache():
    import jax
    try:
        jax.config.update("jax_compilation_cache_dir", "/tmp/jax_kernel_cache")
        jax.config.update("jax_persistent_cache_min_entry_size_bytes", 0)
        jax.config.update("jax_persistent_cache_min_compile_time_secs", 0)
    except Exception:
        pass


G, K, CG, H, W = 4, 9, 16, 128, 128
HP, WP = H + 4, W + 4
PX = HP * WP            # 17424
BASE = WP + 1           # 133
SLACK = 2 * WP + 2      # 266
PXE = PX + 2 * SLACK    # 17956
CHUNK = 484
NCH = PX // CHUNK       # 36
MMF = 484
XRW = CHUNK + 2 * BASE  # 750
EPS = 1e-5
NPIX = H * W

# packed input layout: [64, XCOLS] bf16: raw x image + consts for both
# group-pairs; the zero-extended padded grid is built on device in DRAM
C_CB = [NPIX, NPIX + 260]          # per-gp cstb as 2x[64,130]
C_FH = [NPIX + 520, NPIX + 536]    # per-gp cstf hi [36,8]
C_FL = [NPIX + 528, NPIX + 544]    # per-gp cstf lo [36,8]
XCOLS = NPIX + 552                 # 16936
OCOLS = NPIX + 6

_CACHE = {}


def _build_nc():
    import concourse.bass as bass
    import concourse.mybir as mybir
    from concourse import bacc, tile

    f32 = mybir.dt.float32
    mdt = mybir.dt.bfloat16
    AF = mybir.ActivationFunctionType
    OP = mybir.AluOpType
    AX = mybir.AxisListType

    nc = bacc.Bacc("TRN2", target_bir_lowering=False, debug=False)
    xin = nc.dram_tensor("xin", [64, XCOLS], mdt, kind="ExternalInput")
    outp = nc.dram_tensor("outp", [64, OCOLS], mdt, kind="ExternalOutput")

    ROWS = [128, 128, 48]

    with ExitStack() as ctx:
        tc = ctx.enter_context(tile.TileContext(nc))
        ppool = ctx.enter_context(tc.tile_pool(name="psum", bufs=4,
                                               space="PSUM"))
        dpool = ctx.enter_context(tc.tile_pool(name="drsc", bufs=1,
                                               space="DRAM"))
        # build the zero-extended padded grid [64, PXE] in DRAM scratch:
        # zero-fill, then one strided DMA placing the raw image at
        # (row+2, col+2) of the 132x132 grid, offset by SLACK
        xpe = dpool.tile([64, PXE], mdt, name="xpe")
        with tc.tile_pool(name="zf", bufs=1) as zf:
            zt = zf.tile([64, 2048], mdt, name="zt")
            nc.vector.memset(zt[:], 0.0)
            for c0 in range(0, PXE, 2048):
                w = min(2048, PXE - c0)
                nc.sync.dma_start(xpe[:, c0:c0 + w], zt[:, 0:w])
            nc.sync.dma_start(
                xpe[:, SLACK:SLACK + PX].rearrange(
                    "p (h w) -> p h w", w=WP)[:, 2:2 + H, 2:2 + W],
                xin[:, 0:NPIX].rearrange("p (h w) -> p h w", w=W))
        for gp in range(2):
          with tc.tile_pool(name=f"keep{gp}", bufs=1) as keep:
            rb = gp * 32     # row base of this group-pair's channels in xin
            sb_cb = keep.tile([128, 130], mdt, name=f"cb{gp}")
            nc.sync.dma_start(sb_cb[0:64, :],
                              xin[:, C_CB[gp]:C_CB[gp] + 130])
            nc.sync.dma_start(sb_cb[64:128, :],
                              xin[:, C_CB[gp] + 130:C_CB[gp] + 260])
            cfh = keep.tile([36, 8], mdt, name=f"cfh{gp}")
            cfl = keep.tile([36, 8], mdt, name=f"cfl{gp}")
            nc.sync.dma_start(cfh[:], xin[0:36, C_FH[gp]:C_FH[gp] + 8])
            nc.sync.dma_start(cfl[:], xin[0:36, C_FL[gp]:C_FL[gp] + 8])
            sb_cf = keep.tile([36, 8], f32, name=f"cf{gp}")
            nc.vector.tensor_tensor(sb_cf[:], cfh[:], cfl[:], OP.add)

            sb_womT = sb_cb[0:64, 0:82]
            sb_wA = [sb_cb[:, 82:98], sb_cb[:, 98:114]]
            sb_wB = sb_cb[0:48, 114:130]
            sb_bomYX = sb_cf[0:36, 0:1]
            sb_bomM = sb_cf[0:18, 1:2]
            sb_dcnb = [sb_cf[0:16, 2:3], sb_cf[0:16, 3:4]]
            sb_gnwf = [sb_cf[0:16, 4:5], sb_cf[0:16, 5:6]]
            sb_gnbf = [sb_cf[0:16, 6:7], sb_cf[0:16, 7:8]]

            vsb = [keep.tile([16, PX], mdt, tag=f"vsb{gp}{g}",
                             name=f"vsb{gp}{g}") for g in range(2)]
            moffa = keep.tile([36, 1], f32, name=f"moffa{gp}")
            nc.vector.memset(moffa[:], 0.0)

            with tc.tile_pool(name=f"p2_{gp}", bufs=2) as p2:
                for c in range(NCH):
                    q = c * CHUNK
                    xq = p2.tile([64, CHUNK + 2 * SLACK], mdt, tag="xq",
                                 name=f"xq{gp}")
                    nc.sync.dma_start(xq[:],
                                      xpe[:, q:q + CHUNK + 2 * SLACK])
                    omYX = p2.tile([36, CHUNK], f32, tag="omYX",
                                   name=f"omYX{gp}")
                    omM = p2.tile([18, CHUNK], f32, tag="omM",
                                  name=f"omM{gp}")
                    for s in range(CHUNK // MMF):
                        sl = slice(SLACK + s * MMF, SLACK + (s + 1) * MMF)
                        so = slice(s * MMF, (s + 1) * MMF)
                        ps = ppool.tile([82, MMF], f32, tag="omps",
                                        name=f"omps{gp}")
                        nc.tensor.matmul(ps[:], sb_womT, xq[:, sl],
                                         start=True, stop=True)
                        nc.scalar.activation(omYX[:, so], ps[0:36, :],
                                             AF.Identity, bias=sb_bomYX)
                        nc.scalar.activation(omM[:, so], ps[64:82, :],
                                             AF.Identity, bias=sb_bomM)
                    ayx = p2.tile([36, 3, CHUNK], mdt, tag="ayx",
                                  name=f"ayx{gp}")
                    for m in range(3):
                        tmp = p2.tile([36, CHUNK], f32, tag="tmp_m",
                                      name=f"tmp{gp}")
                        tabs = p2.tile([36, CHUNK], f32, tag="tabs_m",
                                       name=f"tabs{gp}")
                        nc.vector.tensor_scalar(tmp[:], omYX[:],
                                                float(1 - m), None, OP.add)
                        nc.vector.scalar_tensor_tensor(tabs[:], tmp[:], -1.0,
                                                       tmp[:], OP.mult,
                                                       OP.max)
                        if m == 1:
                            mr = p2.tile([36, 1], f32, tag="mr",
                                         name=f"mr{gp}")
                            nc.vector.tensor_reduce(mr[:], tabs[:],
                                                    axis=AX.X, op=OP.max)
                            nc.vector.tensor_tensor(moffa[:], moffa[:],
                                                    mr[:], OP.max)
                        nc.scalar.activation(ayx[:, m, :], tabs[:], AF.Relu,
                                             bias=1.0, scale=-1.0)
                    ms = p2.tile([18, CHUNK], mdt, tag="ms", name=f"ms{gp}")
                    nc.scalar.activation(ms[:], omM[:], AF.Sigmoid)
                    axT = p2.tile([18, 3, CHUNK], mdt, tag="axT",
                                  name=f"axT{gp}")
                    nc.sync.dma_start(axT[:], ayx[18:36, :, :])
                    ayp = p2.tile([18, 3, CHUNK], mdt, tag="ayp",
                                  name=f"ayp{gp}")
                    for m in range(3):
                        nc.vector.tensor_tensor(ayp[:, m, :],
                                                ayx[0:18, m, :], ms[:],
                                                OP.mult)
                    a9p = p2.tile([18, K, CHUNK], mdt, tag="a9p",
                                  name=f"a9p{gp}")
                    for t in range(K):
                        m, n = t // 3, t % 3
                        nc.vector.tensor_tensor(a9p[:, t, :], ayp[:, m, :],
                                                axT[:, n, :], OP.mult)
                    # x replicas: row c*8+k <- xq[rb + gi*16 + c] shifted
                    xrc = [p2.tile([ROWS[i], XRW], mdt, tag=f"xrc{i}",
                                   name=f"xrc{gp}{i}") for i in range(3)]
                    for gi in range(2):
                        xv = xrc[gi][:].rearrange("(c k) w -> k c w", k=8)
                        for k in range(8):
                            ky, kx = k // 3, k % 3
                            s0 = SLACK - BASE + (ky - 1) * WP + (kx - 1)
                            nc.sync.dma_start(
                                xv[k],
                                xq[rb + gi * 16:rb + gi * 16 + 16,
                                   s0:s0 + XRW])
                    s8 = SLACK - BASE + WP + 1
                    nc.sync.dma_start(xrc[2][0:16, :],
                                      xq[rb:rb + 16, s8:s8 + XRW])
                    nc.sync.dma_start(xrc[2][16:32, :],
                                      xq[rb:rb + 16, s8:s8 + XRW])
                    nc.sync.dma_start(xrc[2][32:48, :],
                                      xq[rb + 16:rb + 32, s8:s8 + XRW])
                    # modulation replicas (log-doubling)
                    a9r = [p2.tile([ROWS[i], K, CHUNK], mdt, tag=f"a9r{i}",
                                   name=f"a9r{gp}{i}") for i in range(3)]
                    for g in range(2):
                        nc.sync.dma_start(a9r[g][0:8], a9p[g * 9:g * 9 + 8])
                        nc.sync.dma_start(a9r[g][8:16], a9r[g][0:8])
                        nc.sync.dma_start(a9r[g][16:32], a9r[g][0:16])
                        nc.sync.dma_start(a9r[g][32:64], a9r[g][0:32])
                        nc.sync.dma_start(a9r[g][64:128], a9r[g][0:64])
                        r0 = g * 32
                        nc.sync.dma_start(a9r[2][r0:r0+1],
                                          a9p[g * 9 + 8:g * 9 + 9])
                        nc.sync.dma_start(a9r[2][r0+1:r0+2],
                                          a9r[2][r0:r0+1])
                        nc.sync.dma_start(a9r[2][r0+2:r0+4],
                                          a9r[2][r0:r0+2])
                        nc.sync.dma_start(a9r[2][r0+4:r0+8],
                                          a9r[2][r0:r0+4])
                        nc.sync.dma_start(a9r[2][r0+8:r0+16],
                                          a9r[2][r0:r0+8])
                    nc.sync.dma_start(a9r[2][16:32], a9r[2][0:16])
                    val = [p2.tile([ROWS[i], CHUNK], mdt, tag=f"val{i}",
                                   name=f"val{gp}{i}") for i in range(3)]
                    for i in range(3):
                        prod = p2.tile([ROWS[i], CHUNK], mdt, tag=f"prod{i}",
                                       name=f"prod{gp}{i}")
                        for t in range(K):
                            m, n = t // 3, t % 3
                            off = BASE + (m - 1) * WP + (n - 1)
                            dst = val[i] if t == 0 else prod
                            nc.vector.tensor_tensor(
                                dst[:], a9r[i][:, t, :],
                                xrc[i][:, off:off + CHUNK], OP.mult)
                            if t > 0:
                                nc.vector.tensor_tensor(val[i][:], val[i][:],
                                                        prod[:], OP.add)
                    for g in range(2):
                        for s in range(CHUNK // MMF):
                            sl = slice(s * MMF, (s + 1) * MMF)
                            psv = ppool.tile([16, MMF], f32, tag="psv",
                                             name=f"psv{gp}")
                            nc.tensor.matmul(psv[:], sb_wA[g],
                                             val[g][:, sl],
                                             start=True, stop=False)
                            nc.tensor.matmul(psv[:], sb_wB[g*32:g*32+16, :],
                                             val[2][g*32:g*32+16, sl],
                                             start=False, stop=True)
                            nc.scalar.activation(
                                vsb[g][:, q+s*MMF:q+(s+1)*MMF],
                                psv[:], AF.Identity, bias=sb_dcnb[g])
            moffb = keep.tile([36, 1], mdt, name=f"moffb{gp}")
            nc.vector.tensor_copy(moffb[:], moffa[:])
            nc.sync.dma_start(outp[0:36, NPIX + 4 + gp:NPIX + 5 + gp],
                              moffb[:])

            # GroupNorm + Gelu for this group-pair
            with tc.tile_pool(name=f"p3_{gp}", bufs=1) as p3:
                VOFF = 2 * WP
                invN = 1.0 / (CG * NPIX)
                zero16 = p3.tile([1, 16], f32, name=f"z16{gp}")
                nc.vector.memset(zero16[:], 0.0)
                for g in range(2):
                    orow = gp * 32 + g * 16
                    vg = vsb[g]
                    vap = vg[:, VOFF:VOFF + H*WP].rearrange(
                        "p (h w) -> p h w", w=WP)[:, :, 2:2 + W]
                    vsq = p3.tile([16, PX], f32, tag="vsq",
                                  name=f"vsq{gp}")
                    nc.scalar.activation(vsq[:], vg[:], AF.Square)
                    sqap = vsq[:, VOFF:VOFF + H*WP].rearrange(
                        "p (h w) -> p h w", w=WP)[:, :, 2:2 + W]
                    r1 = p3.tile([16, H], f32, tag="r1", name=f"r1{gp}")
                    s1 = p3.tile([16, 1], f32, tag="s1", name=f"s1{gp}")
                    nc.vector.tensor_reduce(r1[:], vap, axis=AX.X, op=OP.add)
                    nc.vector.tensor_reduce(s1[:], r1[:], axis=AX.X,
                                            op=OP.add)
                    r2 = p3.tile([16, H], f32, tag="r2", name=f"r2{gp}")
                    s2 = p3.tile([16, 1], f32, tag="s2", name=f"s2{gp}")
                    nc.vector.tensor_reduce(r2[:], sqap, axis=AX.X,
                                            op=OP.add)
                    nc.vector.tensor_reduce(s2[:], r2[:], axis=AX.X,
                                            op=OP.add)
                    stats = p3.tile([16, 2], f32, tag="stats",
                                    name=f"stats{gp}")
                    nc.vector.tensor_copy(stats[:, 0:1], s1[:])
                    nc.vector.tensor_copy(stats[:, 1:2], s2[:])
                    sthi = p3.tile([16, 2], mdt, tag="sthi",
                                   name=f"sthi{gp}")
                    nc.vector.tensor_copy(sthi[:], stats[:])
                    sthf = p3.tile([16, 2], f32, tag="sthf",
                                   name=f"sthf{gp}")
                    nc.vector.tensor_copy(sthf[:], sthi[:])
                    stlf = p3.tile([16, 2], f32, tag="stlf",
                                   name=f"stlf{gp}")
                    nc.vector.tensor_tensor(stlf[:], stats[:], sthf[:],
                                            OP.subtract)
                    stlo = p3.tile([16, 2], mdt, tag="stlo",
                                   name=f"stlo{gp}")
                    nc.vector.tensor_copy(stlo[:], stlf[:])
                    nc.sync.dma_start(outp[orow:orow+16, NPIX:NPIX + 2],
                                      sthi[:])
                    nc.sync.dma_start(outp[orow:orow+16, NPIX + 2:NPIX + 4],
                                      stlo[:])
                    scr1 = dpool.tile([16, 1], f32, tag="scr1",
                                      name=f"scr1{gp}")
                    scr2 = dpool.tile([16, 1], f32, tag="scr2",
                                      name=f"scr2{gp}")
                    nc.sync.dma_start(scr1[:], s1[:])
                    nc.sync.dma_start(scr2[:], s2[:])
                    s1t = p3.tile([1, 16], f32, tag="s1t", name=f"s1t{gp}")
                    s2t = p3.tile([1, 16], f32, tag="s2t", name=f"s2t{gp}")
                    nc.sync.dma_start(s1t[:], scr1[:].rearrange("p x -> x p"))
                    nc.sync.dma_start(s2t[:], scr2[:].rearrange("p x -> x p"))
                    mug = p3.tile([1, 1], f32, tag="mug", name=f"mug{gp}")
                    e2g = p3.tile([1, 1], f32, tag="e2g", name=f"e2g{gp}")
                    nc.vector.tensor_reduce(mug[:], s1t[:], axis=AX.X,
                                            op=OP.add)
                    nc.vector.tensor_reduce(e2g[:], s2t[:], axis=AX.X,
                                            op=OP.add)
                    nc.vector.tensor_scalar(mug[:], mug[:], invN, None,
                                            OP.mult)
                    nc.vector.tensor_scalar(e2g[:], e2g[:], invN, None,
                                            OP.mult)
                    var = p3.tile([1, 1], f32, tag="var", name=f"var{gp}")
                    nc.vector.tensor_tensor(var[:], mug[:], mug[:], OP.mult)
                    nc.vector.tensor_tensor(var[:], e2g[:], var[:],
                                            OP.subtract)
                    nc.vector.tensor_scalar(var[:], var[:], EPS, None,
                                            OP.add)
                    sd = p3.tile([1, 1], f32, tag="sd", name=f"sd{gp}")
                    nc.scalar.activation(sd[:], var[:], AF.Sqrt)
                    ivg = p3.tile([1, 1], f32, tag="ivg", name=f"ivg{gp}")
                    nc.vector.reciprocal(ivg[:], sd[:])
                    inv16 = p3.tile([1, 16], f32, tag="inv16",
                                    name=f"inv16{gp}")
                    mu16 = p3.tile([1, 16], f32, tag="mu16",
                                   name=f"mu16{gp}")
                    nc.scalar.activation(inv16[:], zero16[:], AF.Identity,
                                         bias=ivg[:])
                    nc.scalar.activation(mu16[:], zero16[:], AF.Identity,
                                         bias=mug[:])
                    ivp = p3.tile([16, 1], f32, tag="ivp", name=f"ivp{gp}")
                    mup = p3.tile([16, 1], f32, tag="mup", name=f"mup{gp}")
                    scr3 = dpool.tile([1, 16], f32, tag="scr3",
                                      name=f"scr3{gp}")
                    scr4 = dpool.tile([1, 16], f32, tag="scr4",
                                      name=f"scr4{gp}")
                    nc.sync.dma_start(scr3[:], inv16[:])
                    nc.sync.dma_start(scr4[:], mu16[:])
                    nc.sync.dma_start(ivp[:], scr3[:].rearrange("x p -> p x"))
                    nc.sync.dma_start(mup[:], scr4[:].rearrange("x p -> p x"))
                    scp = p3.tile([16, 1], f32, tag="scp", name=f"scp{gp}")
                    bip = p3.tile([16, 1], f32, tag="bip", name=f"bip{gp}")
                    nc.vector.tensor_tensor(scp[:], sb_gnwf[g], ivp[:],
                                            OP.mult)
                    nc.vector.tensor_tensor(bip[:], mup[:], scp[:], OP.mult)
                    nc.vector.tensor_tensor(bip[:], sb_gnbf[g], bip[:],
                                            OP.subtract)
                    og = p3.tile([16, PX], mdt, tag="og", name=f"og{gp}")
                    nc.scalar.activation(og[:], vg[:], AF.Gelu,
                                         bias=bip[:], scale=scp[:])
                    ogap = og[:, VOFF:VOFF + H*WP].rearrange(
                        "p (h w) -> p h w", w=WP)[:, :, 2:2 + W]
                    nc.sync.dma_start(
                        outp[orow:orow+16, 0:NPIX].rearrange(
                            "p (h w) -> p h w", w=W),
                        ogap)

    if not nc.is_finalized():
        nc.finalize()
    return nc


def get_nc():
    _enable_jax_cache()
    if "nc" not in _CACHE:
        nc = _build_nc()
        _json = nc.to_json_bytes()
        nc.to_json_bytes = lambda: _json
        _CACHE["nc"] = nc
    return _CACHE["nc"]


def _consts_for_gp(w_om, b_om, dcn_w, dcn_b, gn_w, gn_b, sc, gp):
    import ml_dtypes
    bf16 = ml_dtypes.bfloat16
    idx_oy = [g * 27 + 2 * k for g in range(G) for k in range(K)]
    idx_ox = [g * 27 + 2 * k + 1 for g in range(G) for k in range(K)]
    idx_ml = [g * 27 + 18 + k for g in range(G) for k in range(K)]
    gsel = [2 * gp, 2 * gp + 1]
    cols = []
    for idx in (idx_oy, idx_ox, idx_ml):
        for g in gsel:
            cols += idx[g * K:(g + 1) * K]
    wsel = w_om[cols].astype(np.float32).copy()
    bsel = b_om[cols].astype(np.float32).copy()
    wsel[:36] *= sc
    bsel[:36] *= sc
    cb = np.zeros((128, 130), np.float32)
    cb[0:64, 0:82] = np.concatenate(
        [wsel[0:36], np.zeros((28, 64), np.float32), wsel[36:54]]).T
    for gi in range(2):
        wg = dcn_w[gsel[gi]].reshape(CG, CG, K)
        cb[:, 82+16*gi:98+16*gi] = np.transpose(
            wg[:, :, :8], (1, 2, 0)).reshape(128, CG)
        cb[gi*32:gi*32+16, 114:130] = wg[:, :, 8].T
    cf = np.zeros((36, 8), np.float32)
    cf[0:36, 0] = bsel[0:36]
    cf[0:18, 1] = bsel[36:54]
    cf[0:16, 2] = dcn_b[gsel[0]]
    cf[0:16, 3] = dcn_b[gsel[1]]
    c0 = gsel[0] * CG
    cf[0:16, 4] = gn_w[c0:c0+16]
    cf[0:16, 5] = gn_w[c0+16:c0+32]
    cf[0:16, 6] = gn_b[c0:c0+16]
    cf[0:16, 7] = gn_b[c0+16:c0+32]
    cf_hi = cf.astype(bf16)
    cf_lo = (cf - cf_hi.astype(np.float32)).astype(bf16)
    return cb.astype(bf16), cf_hi, cf_lo


def _host_prep(x, w_om, b_om, dcn_w, dcn_b, gn_w, gn_b, offset_scale):
    import ml_dtypes
    bf16 = ml_dtypes.bfloat16
    B = x.shape[0]
    sc = float(np.asarray(offset_scale).reshape(-1)[0])
    xb = x.reshape(B, 64, NPIX).astype(bf16)
    consts = [_consts_for_gp(w_om, b_om, dcn_w, dcn_b, gn_w, gn_b, sc, gp)
              for gp in range(2)]
    in_maps = []
    for b in range(B):
        xin = np.zeros((64, XCOLS), bf16)
        xin[:, 0:NPIX] = xb[b]
        for gp in range(2):
            cb, cf_hi, cf_lo = consts[gp]
            xin[:, C_CB[gp]:C_CB[gp] + 130] = cb[0:64]
            xin[:, C_CB[gp] + 130:C_CB[gp] + 260] = cb[64:128]
            xin[0:36, C_FH[gp]:C_FH[gp] + 8] = cf_hi
            xin[0:36, C_FL[gp]:C_FL[gp] + 8] = cf_lo
        in_maps.append({"xin": xin})
    return in_maps


def kernel(x, w_om, b_om, dcn_w, dcn_b, gn_w, gn_b, offset_scale):
    from concourse.bass_utils import run_bass_kernel_spmd

    x = np.asarray(x, np.float32)
    w_om = np.asarray(w_om, np.float32)
    b_om = np.asarray(b_om, np.float32)
    dcn_w = np.asarray(dcn_w, np.float32)
    dcn_b = np.asarray(dcn_b, np.float32)
    gn_w = np.asarray(gn_w, np.float32)
    gn_b = np.asarray(gn_b, np.float32)
    offset_scale = np.asarray(offset_scale, np.float32)
    in_maps = _host_prep(x, w_om, b_om, dcn_w, dcn_b, gn_w, gn_b,
                         offset_scale)
    nc = get_nc()
    res = run_bass_kernel_spmd(nc, in_maps, core_ids=[0, 1, 2, 3])
    out = np.zeros((4, 64, H, W), np.float32)
    stats = np.zeros((4, 64, 2), np.float32)
    moff_all = 0.0
    for b in range(4):
        r = res.results[b]["outp"].astype(np.float32)
        out[b] = r[:, 0:NPIX].reshape(64, H, W)
        stats[b] = r[:, NPIX:NPIX + 2] + r[:, NPIX + 2:NPIX + 4]
        moff_all = max(moff_all, float(r[0:36, NPIX + 4].max()),
                       float(r[0:36, NPIX + 5].max()))
    if moff_all > 0.99:
        out = _host_correct(out, stats, x, w_om, b_om, dcn_w, dcn_b,
                            gn_w, gn_b, offset_scale)
    return out


def _host_correct(out, stats, x, w_om, b_om, dcn_w, dcn_b, gn_w, gn_b,
                  offset_scale):
    """Exact fix for rare pixels with |offset| > 1 (clamped-tri mismatch)."""
    from scipy.special import erf, expit
    sc = float(np.asarray(offset_scale).reshape(-1)[0])
    B = x.shape[0]
    om = (np.einsum('bcp,oc->bop', x.reshape(B, 64, NPIX), w_om)
          + b_om[None, :, None]).reshape(B, 108, H, W)
    for b in range(B):
        for g in range(G):
            oy = om[b, g*27:g*27+18:2] * sc
            ox = om[b, g*27+1:g*27+18:2] * sc
            bad = (np.abs(oy) > 1).any(0) | (np.abs(ox) > 1).any(0)
            if not bad.any():
                continue
            ml = expit(om[b, g*27+18:g*27+27])
            N = CG * NPIX
            mu = stats[b, g*16:g*16+16, 0].sum() / N
            var = stats[b, g*16:g*16+16, 1].sum() / N - mu * mu
            inv = 1.0 / np.sqrt(var + EPS)
            wg = dcn_w[g].reshape(CG, CG, K)
            for hh, ww in zip(*np.nonzero(bad)):
                val = np.zeros((CG, K), np.float32)
                for k in range(K):
                    ky, kx = k // 3, k % 3
                    py = hh + ky - 1 + oy[k, hh, ww]
                    pxx = ww + kx - 1 + ox[k, hh, ww]
                    y0, x0 = int(np.floor(py)), int(np.floor(pxx))
                    fy, fx = py - y0, pxx - x0
                    acc = np.zeros(CG, np.float32)
                    for dy, wy in ((0, 1 - fy), (1, fy)):
                        for dx, wx in ((0, 1 - fx), (1, fx)):
                            yy, xx = y0 + dy, x0 + dx
                            if 0 <= yy < H and 0 <= xx < W:
                                acc += wy * wx * x[b, g*CG:g*CG+CG, yy, xx]
                    val[:, k] = acc * ml[k, hh, ww]
                pre = np.einsum('ck,ock->o', val, wg) + dcn_b[g]
                z = ((pre - mu) * inv * gn_w[g*CG:g*CG+CG]
                     + gn_b[g*CG:g*CG+CG])
                out[b, g*CG:g*CG+CG, hh, ww] = (
                    z * 0.5 * (1.0 + erf(z / np.sqrt(2.0))))
    return out


# revision 7
# speedup vs baseline: 1.0436x; 1.0116x over previous
"""DCNv3 Trainium2 kernel: 4-core SPMD, core = batch (data-parallel over
batch, per the sharding hint). Each core runs both group-pairs
sequentially, reusing SBUF. vs the 8-core variant this halves the
number of transfer shards (per-shard tunnel latency dominates) and
removes the 2x duplication of x across group-pair cores.

Single packed bf16 input per core (padded x grid + weights, f32 consts
as bf16 hi/lo pairs); single packed bf16 output (values + GroupNorm
stats hi/lo + max-offset flags). Shifted x replicas built on device.
Exact for |offset| <= 1; host applies exact correction for larger.
"""
import os
import sys
import numpy as np
from contextlib import ExitStack

for _p in ("/opt/trn_rl_repo",):
    if _p not in sys.path:
        sys.path.insert(0, _p)

os.environ.setdefault("JAX_COMPILATION_CACHE_DIR", "/tmp/jax_kernel_cache")
os.environ.setdefault("JAX_PERSISTENT_CACHE_MIN_ENTRY_SIZE_BYTES", "0")
os.environ.setdefault("JAX_PERSISTENT_CACHE_MIN_COMPILE_TIME_SECS", "0")


def _enable_jax_cache():
    import jax
    try:
        jax.config.update("jax_compilation_cache_dir", "/tmp/jax_kernel_cache")
        jax.config.update("jax_persistent_cache_min_entry_size_bytes", 0)
        jax.config.update("jax_persistent_cache_min_compile_time_secs", 0)
    except Exception:
        pass


G, K, CG, H, W = 4, 9, 16, 128, 128
HP, WP = H + 4, W + 4
PX = HP * WP            # 17424
BASE = WP + 1           # 133
SLACK = 2 * WP + 2      # 266
PXE = PX + 2 * SLACK    # 17956
CHUNK = 484
NCH = PX // CHUNK       # 36
MMF = 484
XRW = CHUNK + 2 * BASE  # 750
EPS = 1e-5
NPIX = H * W

# packed input layout: [64, XCOLS] bf16: raw x image + consts for both
# group-pairs; the zero-extended padded grid is built on device in DRAM
C_CB = [NPIX, NPIX + 260]          # per-gp cstb as 2x[64,130]
C_FH = [NPIX + 520, NPIX + 536]    # per-gp cstf hi [36,8]
C_FL = [NPIX + 528, NPIX + 544]    # per-gp cstf lo [36,8]
XCOLS = NPIX + 552                 # 16936
OCOLS = NPIX + 6

_CACHE = {}


def _build_nc():
    import concourse.bass as bass
    import concourse.mybir as mybir
    from concourse import bacc, tile

    f32 = mybir.dt.float32
    mdt = mybir.dt.bfloat16
    AF = mybir.ActivationFunctionType
    OP = mybir.AluOpType
    AX = mybir.AxisListType

    nc = bacc.Bacc("TRN2", target_bir_lowering=False, debug=False)
    xin = nc.dram_tensor("xin", [64, XCOLS], mdt, kind="ExternalInput")
    outp = nc.dram_tensor("outp", [64, OCOLS], mdt, kind="ExternalOutput")

    ROWS = [128, 128, 48]

    with ExitStack() as ctx:
        tc = ctx.enter_context(tile.TileContext(nc))
        ppool = ctx.enter_context(tc.tile_pool(name="psum", bufs=4,
                                               space="PSUM"))
        dpool = ctx.enter_context(tc.tile_pool(name="drsc", bufs=1,
                                               space="DRAM"))
        # build the zero-extended padded grid [64, PXE] in DRAM scratch:
        # zero-fill, then one strided DMA placing the raw image at
        # (row+2, col+2) of the 132x132 grid, offset by SLACK
        xpe = dpool.tile([64, PXE], mdt, name="xpe")
        with tc.tile_pool(name="zf", bufs=1) as zf:
            zt = zf.tile([64, 2048], mdt, name="zt")
            nc.vector.memset(zt[:], 0.0)
            for c0 in range(0, PXE, 2048):
                w = min(2048, PXE - c0)
                nc.sync.dma_start(xpe[:, c0:c0 + w], zt[:, 0:w])
            nc.sync.dma_start(
                xpe[:, SLACK:SLACK + PX].rearrange(
                    "p (h w) -> p h w", w=WP)[:, 2:2 + H, 2:2 + W],
                xin[:, 0:NPIX].rearrange("p (h w) -> p h w", w=W))
        for gp in range(2):
          with tc.tile_pool(name=f"keep{gp}", bufs=1) as keep:
            rb = gp * 32     # row base of this group-pair's channels in xin
            sb_cb = keep.tile([128, 130], mdt, name=f"cb{gp}")
            nc.sync.dma_start(sb_cb[0:64, :],
                              xin[:, C_CB[gp]:C_CB[gp] + 130])
            nc.sync.dma_start(sb_cb[64:128, :],
                              xin[:, C_CB[gp] + 130:C_CB[gp] + 260])
            cfh = keep.tile([36, 8], mdt, name=f"cfh{gp}")
            cfl = keep.tile([36, 8], mdt, name=f"cfl{gp}")
            nc.sync.dma_start(cfh[:], xin[0:36, C_FH[gp]:C_FH[gp] + 8])
            nc.sync.dma_start(cfl[:], xin[0:36, C_FL[gp]:C_FL[gp] + 8])
            sb_cf = keep.tile([36, 8], f32, name=f"cf{gp}")
            nc.vector.tensor_tensor(sb_cf[:], cfh[:], cfl[:], OP.add)

            sb_womT = sb_cb[0:64, 0:82]
            sb_wA = [sb_cb[:, 82:98], sb_cb[:, 98:114]]
            sb_wB = sb_cb[0:48, 114:130]
            sb_bomYX = sb_cf[0:36, 0:1]
            sb_bomM = sb_cf[0:18, 1:2]
            sb_dcnb = [sb_cf[0:16, 2:3], sb_cf[0:16, 3:4]]
            sb_gnwf = [sb_cf[0:16, 4:5], sb_cf[0:16, 5:6]]
            sb_gnbf = [sb_cf[0:16, 6:7], sb_cf[0:16, 7:8]]

            vsb = [keep.tile([16, PX], mdt, tag=f"vsb{gp}{g}",
                             name=f"vsb{gp}{g}") for g in range(2)]
            moffa = keep.tile([36, 1], f32, name=f"moffa{gp}")
            nc.vector.memset(moffa[:], 0.0)

            with tc.tile_pool(name=f"p2_{gp}", bufs=2) as p2:
                for c in range(NCH):
                    q = c * CHUNK
                    xq = p2.tile([64, CHUNK + 2 * SLACK], mdt, tag="xq",
                                 name=f"xq{gp}")
                    nc.sync.dma_start(xq[:],
                                      xpe[:, q:q + CHUNK + 2 * SLACK])
                    omYX = p2.tile([36, CHUNK], f32, tag="omYX",
                                   name=f"omYX{gp}")
                    omM = p2.tile([18, CHUNK], f32, tag="omM",
                                  name=f"omM{gp}")
                    for s in range(CHUNK // MMF):
                        sl = slice(SLACK + s * MMF, SLACK + (s + 1) * MMF)
                        so = slice(s * MMF, (s + 1) * MMF)
                        ps = ppool.tile([82, MMF], f32, tag="omps",
                                        name=f"omps{gp}")
                        nc.tensor.matmul(ps[:], sb_womT, xq[:, sl],
                                         start=True, stop=True)
                        nc.scalar.activation(omYX[:, so], ps[0:36, :],
                                             AF.Identity, bias=sb_bomYX)
                        nc.scalar.activation(omM[:, so], ps[64:82, :],
                                             AF.Identity, bias=sb_bomM)
                    ayx = p2.tile([36, 3, CHUNK], mdt, tag="ayx",
                                  name=f"ayx{gp}")
                    for m in range(3):
                        tmp = p2.tile([36, CHUNK], f32, tag="tmp_m",
                                      name=f"tmp{gp}")
                        tabs = p2.tile([36, CHUNK], f32, tag="tabs_m",
                                       name=f"tabs{gp}")
                        nc.vector.tensor_scalar(tmp[:], omYX[:],
                                                float(1 - m), None, OP.add)
                        nc.vector.scalar_tensor_tensor(tabs[:], tmp[:], -1.0,
                                                       tmp[:], OP.mult,
                                                       OP.max)
                        if m == 1:
                            mr = p2.tile([36, 1], f32, tag="mr",
                                         name=f"mr{gp}")
                            nc.vector.tensor_reduce(mr[:], tabs[:],
                                                    axis=AX.X, op=OP.max)
                            nc.vector.tensor_tensor(moffa[:], moffa[:],
                                                    mr[:], OP.max)
                        nc.scalar.activation(ayx[:, m, :], tabs[:], AF.Relu,
                                             bias=1.0, scale=-1.0)
                    ms = p2.tile([18, CHUNK], mdt, tag="ms", name=f"ms{gp}")
                    nc.scalar.activation(ms[:], omM[:], AF.Sigmoid)
                    axT = p2.tile([18, 3, CHUNK], mdt, tag="axT",
                                  name=f"axT{gp}")
                    nc.sync.dma_start(axT[:], ayx[18:36, :, :])
                    ayp = p2.tile([18, 3, CHUNK], mdt, tag="ayp",
                                  name=f"ayp{gp}")
                    for m in range(3):
                        nc.vector.tensor_tensor(ayp[:, m, :],
                                                ayx[0:18, m, :], ms[:],
                                                OP.mult)
                    a9p = p2.tile([18, K, CHUNK], mdt, tag="a9p",
                                  name=f"a9p{gp}")
                    for t in range(K):
                        m, n = t // 3, t % 3
                        nc.vector.tensor_tensor(a9p[:, t, :], ayp[:, m, :],
                                                axT[:, n, :], OP.mult)
                    # x replicas: row c*8+k <- xq[rb + gi*16 + c] shifted
                    xrc = [p2.tile([ROWS[i], XRW], mdt, tag=f"xrc{i}",
                                   name=f"xrc{gp}{i}") for i in range(3)]
                    for gi in range(2):
                        xv = xrc[gi][:].rearrange("(c k) w -> k c w", k=8)
                        for k in range(8):
                            ky, kx = k // 3, k % 3
                            s0 = SLACK - BASE + (ky - 1) * WP + (kx - 1)
                            nc.sync.dma_start(
                                xv[k],
                                xq[rb + gi * 16:rb + gi * 16 + 16,
                                   s0:s0 + XRW])
                    s8 = SLACK - BASE + WP + 1
                    nc.sync.dma_start(xrc[2][0:16, :],
                                      xq[rb:rb + 16, s8:s8 + XRW])
                    nc.sync.dma_start(xrc[2][16:32, :],
                                      xq[rb:rb + 16, s8:s8 + XRW])
                    nc.sync.dma_start(xrc[2][32:48, :],
                                      xq[rb + 16:rb + 32, s8:s8 + XRW])
                    # modulation replicas (log-doubling)
                    a9r = [p2.tile([ROWS[i], K, CHUNK], mdt, tag=f"a9r{i}",
                                   name=f"a9r{gp}{i}") for i in range(3)]
                    for g in range(2):
                        nc.sync.dma_start(a9r[g][0:8], a9p[g * 9:g * 9 + 8])
                        nc.sync.dma_start(a9r[g][8:16], a9r[g][0:8])
                        nc.sync.dma_start(a9r[g][16:32], a9r[g][0:16])
                        nc.sync.dma_start(a9r[g][32:64], a9r[g][0:32])
                        nc.sync.dma_start(a9r[g][64:128], a9r[g][0:64])
                        r0 = g * 32
                        nc.sync.dma_start(a9r[2][r0:r0+1],
                                          a9p[g * 9 + 8:g * 9 + 9])
                        nc.sync.dma_start(a9r[2][r0+1:r0+2],
                                          a9r[2][r0:r0+1])
                        nc.sync.dma_start(a9r[2][r0+2:r0+4],
                                          a9r[2][r0:r0+2])
                        nc.sync.dma_start(a9r[2][r0+4:r0+8],
                                          a9r[2][r0:r0+4])
                        nc.sync.dma_start(a9r[2][r0+8:r0+16],
                                          a9r[2][r0:r0+8])
                    nc.sync.dma_start(a9r[2][16:32], a9r[2][0:16])
                    val = [p2.tile([ROWS[i], CHUNK], mdt, tag=f"val{i}",
                                   name=f"val{gp}{i}") for i in range(3)]
                    for i in range(3):
                        prod = p2.tile([ROWS[i], CHUNK], mdt, tag=f"prod{i}",
                                       name=f"prod{gp}{i}")
                        for t in range(K):
                            m, n = t // 3, t % 3
                            off = BASE + (m - 1) * WP + (n - 1)
                            dst = val[i] if t == 0 else prod
                            nc.vector.tensor_tensor(
                                dst[:], a9r[i][:, t, :],
                                xrc[i][:, off:off + CHUNK], OP.mult)
                            if t > 0:
                                nc.vector.tensor_tensor(val[i][:], val[i][:],
                                                        prod[:], OP.add)
                    for g in range(2):
                        for s in range(CHUNK // MMF):
                            sl = slice(s * MMF, (s + 1) * MMF)
                            psv = ppool.tile([16, MMF], f32, tag="psv",
                                             name=f"psv{gp}")
                            nc.tensor.matmul(psv[:], sb_wA[g],
                                             val[g][:, sl],
                                             start=True, stop=False)
                            nc.tensor.matmul(psv[:], sb_wB[g*32:g*32+16, :],
                                             val[2][g*32:g*32+16, sl],
                                             start=False, stop=True)
                            nc.scalar.activation(
                                vsb[g][:, q+s*MMF:q+(s+1)*MMF],
                                psv[:], AF.Identity, bias=sb_dcnb[g])
            moffb = keep.tile([36, 1], mdt, name=f"moffb{gp}")
            nc.vector.tensor_copy(moffb[:], moffa[:])
            nc.sync.dma_start(outp[0:36, NPIX + 4 + gp:NPIX + 5 + gp],
                              moffb[:])

            # GroupNorm + Gelu for this group-pair
            with tc.tile_pool(name=f"p3_{gp}", bufs=1) as p3:
                VOFF = 2 * WP
                invN = 1.0 / (CG * NPIX)
                zero16 = p3.tile([1, 16], f32, name=f"z16{gp}")
                nc.vector.memset(zero16[:], 0.0)
                for g in range(2):
                    orow = gp * 32 + g * 16
                    vg = vsb[g]
                    vap = vg[:, VOFF:VOFF + H*WP].rearrange(
                        "p (h w) -> p h w", w=WP)[:, :, 2:2 + W]
                    vsq = p3.tile([16, PX], f32, tag="vsq",
                                  name=f"vsq{gp}")
                    nc.scalar.activation(vsq[:], vg[:], AF.Square)
                    sqap = vsq[:, VOFF:VOFF + H*WP].rearrange(
                        "p (h w) -> p h w", w=WP)[:, :, 2:2 + W]
                    r1 = p3.tile([16, H], f32, tag="r1", name=f"r1{gp}")
                    s1 = p3.tile([16, 1], f32, tag="s1", name=f"s1{gp}")
                    nc.vector.tensor_reduce(r1[:], vap, axis=AX.X, op=OP.add)
                    nc.vector.tensor_reduce(s1[:], r1[:], axis=AX.X,
                                            op=OP.add)
                    r2 = p3.tile([16, H], f32, tag="r2", name=f"r2{gp}")
                    s2 = p3.tile([16, 1], f32, tag="s2", name=f"s2{gp}")
                    nc.vector.tensor_reduce(r2[:], sqap, axis=AX.X,
                                            op=OP.add)
                    nc.vector.tensor_reduce(s2[:], r2[:], axis=AX.X,
                                            op=OP.add)
                    stats = p3.tile([16, 2], f32, tag="stats",
                                    name=f"stats{gp}")
                    nc.vector.tensor_copy(stats[:, 0:1], s1[:])
                    nc.vector.tensor_copy(stats[:, 1:2], s2[:])
                    sthi = p3.tile([16, 2], mdt, tag="sthi",
                                   name=f"sthi{gp}")
                    nc.vector.tensor_copy(sthi[:], stats[:])
                    sthf = p3.tile([16, 2], f32, tag="sthf",
                                   name=f"sthf{gp}")
                    nc.vector.tensor_copy(sthf[:], sthi[:])
                    stlf = p3.tile([16, 2], f32, tag="stlf",
                                   name=f"stlf{gp}")
                    nc.vector.tensor_tensor(stlf[:], stats[:], sthf[:],
                                            OP.subtract)
                    stlo = p3.tile([16, 2], mdt, tag="stlo",
                                   name=f"stlo{gp}")
                    nc.vector.tensor_copy(stlo[:], stlf[:])
                    nc.sync.dma_start(outp[orow:orow+16, NPIX:NPIX + 2],
                                      sthi[:])
                    nc.sync.dma_start(outp[orow:orow+16, NPIX + 2:NPIX + 4],
                                      stlo[:])
                    scr1 = dpool.tile([16, 1], f32, tag="scr1",
                                      name=f"scr1{gp}")
                    scr2 = dpool.tile([16, 1], f32, tag="scr2",
                                      name=f"scr2{gp}")
                    nc.sync.dma_start(scr1[:], s1[:])
                    nc.sync.dma_start(scr2[:], s2[:])
                    s1t = p3.tile([1, 16], f32, tag="s1t", name=f"s1t{gp}")
                    s2t = p3.tile([1, 16], f32, tag="s2t", name=f"s2t{gp}")
                    nc.sync.dma_start(s1t[:], scr1[:].rearrange("p x -> x p"))
                    nc.sync.dma_start(s2t[:], scr2[:].rearrange("p x -> x p"))
                    mug = p3.tile([1, 1], f32, tag="mug", name=f"mug{gp}")
                    e2g = p3.tile([1, 1], f32, tag="e2g", name=f"e2g{gp}")
                    nc.vector.tensor_reduce(mug[:], s1t[:], axis=AX.X,
                                            op=OP.add)
                    nc.vector.tensor_reduce(e2g[:], s2t[:], axis=AX.X,
                                            op=OP.add)
                    nc.vector.tensor_scalar(mug[:], mug[:], invN, None,
                                            OP.mult)
                    nc.vector.tensor_scalar(e2g[:], e2g[:], invN, None,
                                            OP.mult)
                    var = p3.tile([1, 1], f32, tag="var", name=f"var{gp}")
                    nc.vector.tensor_tensor(var[:], mug[:], mug[:], OP.mult)
                    nc.vector.tensor_tensor(var[:], e2g[:], var[:],
                                            OP.subtract)
                    nc.vector.tensor_scalar(var[:], var[:], EPS, None,
                                            OP.add)
                    sd = p3.tile([1, 1], f32, tag="sd", name=f"sd{gp}")
                    nc.scalar.activation(sd[:], var[:], AF.Sqrt)
                    ivg = p3.tile([1, 1], f32, tag="ivg", name=f"ivg{gp}")
                    nc.vector.reciprocal(ivg[:], sd[:])
                    inv16 = p3.tile([1, 16], f32, tag="inv16",
                                    name=f"inv16{gp}")
                    mu16 = p3.tile([1, 16], f32, tag="mu16",
                                   name=f"mu16{gp}")
                    nc.scalar.activation(inv16[:], zero16[:], AF.Identity,
                                         bias=ivg[:])
                    nc.scalar.activation(mu16[:], zero16[:], AF.Identity,
                                         bias=mug[:])
                    ivp = p3.tile([16, 1], f32, tag="ivp", name=f"ivp{gp}")
                    mup = p3.tile([16, 1], f32, tag="mup", name=f"mup{gp}")
                    scr3 = dpool.tile([1, 16], f32, tag="scr3",
                                      name=f"scr3{gp}")
                    scr4 = dpool.tile([1, 16], f32, tag="scr4",
                                      name=f"scr4{gp}")
                    nc.sync.dma_start(scr3[:], inv16[:])
                    nc.sync.dma_start(scr4[:], mu16[:])
                    nc.sync.dma_start(ivp[:], scr3[:].rearrange("x p -> p x"))
                    nc.sync.dma_start(mup[:], scr4[:].rearrange("x p -> p x"))
                    scp = p3.tile([16, 1], f32, tag="scp", name=f"scp{gp}")
                    bip = p3.tile([16, 1], f32, tag="bip", name=f"bip{gp}")
                    nc.vector.tensor_tensor(scp[:], sb_gnwf[g], ivp[:],
                                            OP.mult)
                    nc.vector.tensor_tensor(bip[:], mup[:], scp[:], OP.mult)
                    nc.vector.tensor_tensor(bip[:], sb_gnbf[g], bip[:],
                                            OP.subtract)
                    og = p3.tile([16, PX], mdt, tag="og", name=f"og{gp}")
                    nc.scalar.activation(og[:], vg[:], AF.Gelu,
                                         bias=bip[:], scale=scp[:])
                    ogap = og[:, VOFF:VOFF + H*WP].rearrange(
                        "p (h w) -> p h w", w=WP)[:, :, 2:2 + W]
                    nc.sync.dma_start(
                        outp[orow:orow+16, 0:NPIX].rearrange(
                            "p (h w) -> p h w", w=W),
                        ogap)

    if not nc.is_finalized():
        nc.finalize()
    return nc


def get_nc():
    _enable_jax_cache()
    if "nc" not in _CACHE:
        nc = _build_nc()
        _json = nc.to_json_bytes()
        nc.to_json_bytes = lambda: _json
        _CACHE["nc"] = nc
    return _CACHE["nc"]


def _consts_for_gp(w_om, b_om, dcn_w, dcn_b, gn_w, gn_b, sc, gp):
    import ml_dtypes
    bf16 = ml_dtypes.bfloat16
    idx_oy = [g * 27 + 2 * k for g in range(G) for k in range(K)]
    idx_ox = [g * 27 + 2 * k + 1 for g in range(G) for k in range(K)]
    idx_ml = [g * 27 + 18 + k for g in range(G) for k in range(K)]
    gsel = [2 * gp, 2 * gp + 1]
    cols = []
    for idx in (idx_oy, idx_ox, idx_ml):
        for g in gsel:
            cols += idx[g * K:(g + 1) * K]
    wsel = w_om[cols].astype(np.float32).copy()
    bsel = b_om[cols].astype(np.float32).copy()
    wsel[:36] *= sc
    bsel[:36] *= sc
    cb = np.zeros((128, 130), np.float32)
    cb[0:64, 0:82] = np.concatenate(
        [wsel[0:36], np.zeros((28, 64), np.float32), wsel[36:54]]).T
    for gi in range(2):
        wg = dcn_w[gsel[gi]].reshape(CG, CG, K)
        cb[:, 82+16*gi:98+16*gi] = np.transpose(
            wg[:, :, :8], (1, 2, 0)).reshape(128, CG)
        cb[gi*32:gi*32+16, 114:130] = wg[:, :, 8].T
    cf = np.zeros((36, 8), np.float32)
    cf[0:36, 0] = bsel[0:36]
    cf[0:18, 1] = bsel[36:54]
    cf[0:16, 2] = dcn_b[gsel[0]]
    cf[0:16, 3] = dcn_b[gsel[1]]
    c0 = gsel[0] * CG
    cf[0:16, 4] = gn_w[c0:c0+16]
    cf[0:16, 5] = gn_w[c0+16:c0+32]
    cf[0:16, 6] = gn_b[c0:c0+16]
    cf[0:16, 7] = gn_b[c0+16:c0+32]
    cf_hi = cf.astype(bf16)
    cf_lo = (cf - cf_hi.astype(np.float32)).astype(bf16)
    return cb.astype(bf16), cf_hi, cf_lo


def _host_prep(x, w_om, b_om, dcn_w, dcn_b, gn_w, gn_b, offset_scale):
    import ml_dtypes
    bf16 = ml_dtypes.bfloat16
    B = x.shape[0]
    sc = float(np.asarray(offset_scale).reshape(-1)[0])
    xb = x.reshape(B, 64, NPIX).astype(bf16)
    consts = [_consts_for_gp(w_om, b_om, dcn_w, dcn_b, gn_w, gn_b, sc, gp)
              for gp in range(2)]
    in_maps = []
    for b in range(B):
        xin = np.zeros((64, XCOLS), bf16)
        xin[:, 0:NPIX] = xb[b]
        for gp in range(2):
            cb, cf_hi, cf_lo = consts[gp]
            xin[:, C_CB[gp]:C_CB[gp] + 130] = cb[0:64]
            xin[:, C_CB[gp] + 130:C_CB[gp] + 260] = cb[64:128]
            xin[0:36, C_FH[gp]:C_FH[gp] + 8] = cf_hi
            xin[0:36, C_FL[gp]:C_FL[gp] + 8] = cf_lo
        in_maps.append({"xin": xin})
    return in_maps


def kernel(x, w_om, b_om, dcn_w, dcn_b, gn_w, gn_b, offset_scale):
    from concourse.bass_utils import run_bass_kernel_spmd

    x = np.asarray(x, np.float32)
    w_om = np.asarray(w_om, np.float32)
    b_om = np.asarray(b_om, np.float32)
    dcn_w = np.asarray(dcn_w, np.float32)
    dcn_b = np.asarray(dcn_b, np.float32)
    gn_w = np.asarray(gn_w, np.float32)
    gn_b = np.asarray(gn_b, np.float32)
    offset_scale = np.asarray(offset_scale, np.float32)
    in_maps = _host_prep(x, w_om, b_om, dcn_w, dcn_b, gn_w, gn_b,
                         offset_scale)
    nc = get_nc()
    res = run_bass_kernel_spmd(nc, in_maps, core_ids=[0, 1, 2, 3])
    out = np.zeros((4, 64, H, W), np.float32)
    stats = np.zeros((4, 64, 2), np.float32)
    moff_all = 0.0
    for b in range(4):
        r = res.results[b]["outp"].astype(np.float32)
        out[b] = r[:, 0:NPIX].reshape(64, H, W)
        stats[b] = r[:, NPIX:NPIX + 2] + r[:, NPIX + 2:NPIX + 4]
        moff_all = max(moff_all, float(r[0:36, NPIX + 4].max()),
                       float(r[0:36, NPIX + 5].max()))
    if moff_all > 0.99:
        # some |offset| > 1: the device's clamped-tri values (and hence
        # the GroupNorm stats) are approximate there - recompute exactly
        out = _host_exact(x, w_om, b_om, dcn_w, dcn_b, gn_w, gn_b,
                          offset_scale)
    return out


def _host_exact(x, w_om, b_om, dcn_w, dcn_b, gn_w, gn_b, offset_scale):
    """Exact f32 recompute of the whole module (rare: only when any
    |offset| > 1, where the device clamped-tri path is approximate)."""
    from scipy.special import erf, expit
    sc = float(np.asarray(offset_scale).reshape(-1)[0])
    B = x.shape[0]
    om = (np.einsum('bcp,oc->bop', x.reshape(B, 64, NPIX), w_om)
          + b_om[None, :, None]).reshape(B, 108, H, W)
    ky = (np.arange(K) // 3).astype(np.float32)
    kx = (np.arange(K) % 3).astype(np.float32)
    hh = np.arange(H, dtype=np.float32)
    ww = np.arange(W, dtype=np.float32)
    outs = []
    for g in range(G):
        x_g = x[:, g * CG:(g + 1) * CG]
        off_y = om[:, g*27:g*27+18:2] * sc
        off_x = om[:, g*27+1:g*27+18:2] * sc
        mask = expit(om[:, g*27+18:g*27+27])
        py = (hh[None, None, :, None] - 1 + ky[None, :, None, None]
              + off_y)
        px = (ww[None, None, None, :] - 1 + kx[None, :, None, None]
              + off_x)
        y0 = np.floor(py); x0 = np.floor(px)
        wy1 = py - y0; wx1 = px - x0
        img_flat = x_g.reshape(B, CG, H * W)

        def gath(yc, xc):
            valid = (yc >= 0) & (yc <= H - 1) & (xc >= 0) & (xc <= W - 1)
            yi = np.clip(yc, 0, H - 1).astype(np.int64)
            xi = np.clip(xc, 0, W - 1).astype(np.int64)
            flat = (yi * W + xi).reshape(B, 1, -1)
            v = np.take_along_axis(img_flat, flat, axis=2).reshape(
                B, CG, K, H, W)
            return v * valid[:, None]

        val = (gath(y0, x0) * ((1-wy1) * (1-wx1))[:, None]
               + gath(y0, x0 + 1) * ((1-wy1) * wx1)[:, None]
               + gath(y0 + 1, x0) * (wy1 * (1-wx1))[:, None]
               + gath(y0 + 1, x0 + 1) * (wy1 * wx1)[:, None])
        val = val * mask[:, None]
        o = np.einsum('bckhw,ock->bohw', val,
                      dcn_w[g].reshape(CG, CG, K))
        outs.append(o + dcn_b[g][None, :, None, None])
    out = np.concatenate(outs, axis=1)
    og = out.reshape(B, G, CG, H, W)
    mu = og.mean(axis=(2, 3, 4), keepdims=True)
    var = og.var(axis=(2, 3, 4), keepdims=True)
    og = (og - mu) / np.sqrt(var + EPS)
    z = (og.reshape(B, 64, H, W) * gn_w[None, :, None, None]
         + gn_b[None, :, None, None])
    return (z * 0.5 * (1.0 + erf(z / np.sqrt(2.0)))).astype(np.float32)
